# revision 39
# baseline (speedup 1.0000x reference)
"""Trainium2 Bass kernel for the ActionSelector GNN-MLP problem.

Model (per node n, graph g = graph of n):
    x      = [node_feat(n) | node_feat(prev(g)) | ctx(g)]   # 320
    h1     = relu(x @ W1 + b1)                              # 256
    h2     = relu(h1 @ W2 + b2)                             # 128
    logits = h2 @ W3 + b3                                   # 1

Strategy: data-parallel over graphs across 8 cores.  Per core the MLP is
decomposed with the per-graph pieces precomputed on the host:
    pgb[g] = prev_feat[g] @ W1b + ctx[g] @ W1c + b1         # [G, 256]
    qgb[g] = pgb[g] @ W2 + b2                               # [G, 128]
and on device (exact algebra, no approximation):
    a   = node_feat @ W1a                 (2 bf16 matmuls)
    h1' = max(a, -pgb[g])                 (= relu(a+pgb) - pgb; fused DVE max
                                           with a stride-0 broadcast operand)
    h2  = relu(h1' @ W2 + qgb[g])         (qgb broadcast via ONE one-hot
                                           selector matmul in h2 space)
    out = h2 @ W3 + b3
qgb lives in 120-graph tiles (10 blocks each) so a block's 12 graphs never
straddle a tile boundary: exactly one selector matmul per block, from a
fixed set of 10 one-hot patterns.  Nodes are contiguous by graph
(40/graph, blocks of 12 graphs).  npgb is chunked progressively so the
first blocks only wait on tiny const DMAs.
"""

import os
import sys

import ml_dtypes
import numpy as np

BF16_NP = ml_dtypes.bfloat16

try:
    import concourse.bass as bass  # noqa: F401
except ImportError:  # harness containers keep the repo here
    sys.path.insert(0, "/opt/trn_rl_repo")

import concourse.bacc as bacc
import concourse.bass as bass
import concourse.mybir as mybir
import concourse.tile as tile
from concourse.bass_utils import run_bass_kernel_spmd

F32 = mybir.dt.float32
BF16 = mybir.dt.bfloat16

P = 128
D = 128          # node feature dim
DCTX = 64
H1 = 256
H2 = 128
NPG = 40         # nodes per graph
N_GRAPHS = 12500
N_NODES = N_GRAPHS * NPG

N_CORES = 8
GPB = 12                   # graphs per block
NB = GPB * NPG             # 480 nodes per block
BLOCKS = 132               # blocks per core
QUADS = BLOCKS // 4
G_PC = BLOCKS * GPB        # 1584 graphs per core (padded)
NODES_PC = BLOCKS * NB     # 63360 nodes per core (padded)
PAIRS = BLOCKS // 2

# npgb (broadcast operand) chunking: progressive sizes so the first
# blocks only wait on tiny DMAs
NCH_BLOCKS = [2, 10, 40, 80]
NCH_G = [b * GPB for b in NCH_BLOCKS]   # graphs per chunk
NCH = len(NCH_BLOCKS)

# qgb tiling: 120 graphs (10 blocks) per 128-partition tile so a block's
# 12 graphs never straddle tiles; block b -> tile b//10, pattern b%10
BPT = 10                       # blocks per qgb tile
GPT = BPT * GPB                # 120 graphs per tile
NQT = (BLOCKS + BPT - 1) // BPT  # 14 qgb tiles
NPAT = BPT                     # 10 selector patterns

# mixed bias routing: every ROUTE_EVERY-th block takes its L2 bias as a
# DVE max (h2' = max(h2ps, -qgb), per-graph qgb@W3 re-added on the host)
# instead of the one-hot selector matmul, trading idle DVE capacity for
# tensor-engine cycles
ROUTE_EVERY = 3


def _is_routed(b):
    # every 3rd block, minus every 9th of those: 39 of 132 blocks, the
    # measured DVE/PE balance point
    return b % ROUTE_EVERY == ROUTE_EVERY - 1 and b % 27 != 2


ROUTED = [b for b in range(BLOCKS) if _is_routed(b)]
N_ROUTED = len(ROUTED)
ROUTED_IDX = {b: r for r, b in enumerate(ROUTED)}

_PROGRAM = None


def _chunk_of(b):
    """(chunk index, local graph offset) for block b."""
    acc = 0
    for k, nb in enumerate(NCH_BLOCKS):
        if b < acc + nb:
            return k, GPB * (b - acc)
        acc += nb
    raise AssertionError(b)


def _build_program():
    nc = bacc.Bacc(None, target_bir_lowering=False, debug=False)

    xt_t = nc.dram_tensor("xt", [PAIRS, P, 2 * NB], BF16, kind="ExternalInput")
    npgb_t = [
        nc.dram_tensor(f"npgb{k}", [P, 2 * g], BF16, kind="ExternalInput")
        for k, g in enumerate(NCH_G)
    ]
    qgb_t = nc.dram_tensor("qgb", [P, NQT * H2], BF16, kind="ExternalInput")
    nqgb_t = nc.dram_tensor("nqgb", [P, N_ROUTED * GPB], BF16, kind="ExternalInput")
    sel_t = nc.dram_tensor("sel", [P, NPAT * NB], BF16, kind="ExternalInput")
    w1a_t = nc.dram_tensor("w1a", [D, H1], BF16, kind="ExternalInput")
    w2_t = nc.dram_tensor("w2", [H1, H2], BF16, kind="ExternalInput")
    w3_t = nc.dram_tensor("w3", [H2, 32], BF16, kind="ExternalInput")
    b3_t = nc.dram_tensor("b3", [P, 1], F32, kind="ExternalInput")
    out_t = nc.dram_tensor("out", [QUADS, 4, NB], F32, kind="ExternalOutput")

    RELU = mybir.ActivationFunctionType.Relu
    IDENT = mybir.ActivationFunctionType.Identity
    MULT = mybir.AluOpType.mult
    MAX = mybir.AluOpType.max

    with tile.TileContext(nc) as tc:
        with (
            tc.tile_pool(name="const", bufs=1) as cp,
            tc.tile_pool(name="work", bufs=3) as wp,
            tc.tile_pool(name="hbuf", bufs=4) as hp,
            tc.tile_pool(name="psum", bufs=2, space="PSUM") as pp,
        ):
            # ---- resident constants -------------------------------------
            w1a_s = cp.tile([D, H1], BF16)
            w2a_s = cp.tile([P, H2], BF16)
            w2b_s = cp.tile([P, H2], BF16)
            w3_s = cp.tile([H2, 32], BF16)
            b3_s = cp.tile([P, 1], F32)
            qgb_s = cp.tile([P, NQT * H2], BF16)
            nqgb_s = cp.tile([P, N_ROUTED * GPB], BF16)
            sel_s = cp.tile([P, NPAT * NB], BF16)
            npgb_s = [
                cp.tile([P, 2 * g], BF16, name=f"npgb_s{k}")
                for k, g in enumerate(NCH_G)
            ]

            # earliest-needed first.  The gpsimd queue is deliberately
            # UNUSED for DMA: its dma path is SWDGE (Q7 software descriptor
            # generation, ~1us fixed overhead per dma and a ~4us drain in
            # the epilogue).  sync carries xt + output; consts are split
            # between the scalar and vector HWDGE queues.
            nc.scalar.dma_start(out=w1a_s[:], in_=w1a_t[:])          # block 0
            nc.scalar.dma_start(out=npgb_s[0][:], in_=npgb_t[0][:])  # block 0
            nc.scalar.dma_start(out=npgb_s[1][:], in_=npgb_t[1][:])  # block 2
            nc.scalar.dma_start(out=w2a_s[:], in_=w2_t[0:P, :])      # iter 3
            nc.scalar.dma_start(out=w2b_s[:], in_=w2_t[P : 2 * P, :])
            nc.scalar.dma_start(                                     # iter 3
                out=sel_s[:, 0 : 2 * NB], in_=sel_t[:, 0 : 2 * NB]
            )
            nc.scalar.dma_start(                                     # iter 3
                out=qgb_s[:, 0 : 2 * H2], in_=qgb_t[:, 0 : 2 * H2]
            )
            nc.scalar.dma_start(out=w3_s[:], in_=w3_t[:])            # iter 6
            nc.scalar.dma_start(out=b3_s[:], in_=b3_t[:])

            # remaining big consts are dispatched inside the loop (below)
            # so they do not contend with the first xt tiles for DMA
            # bandwidth; keyed by iteration, comfortably before first use
            deferred = {
                1: lambda: nc.scalar.dma_start(        # iter 8
                    out=nqgb_s[:], in_=nqgb_t[:]
                ),
                2: lambda: nc.scalar.dma_start(        # patterns 2-4, block 2
                    out=sel_s[:, 2 * NB : 5 * NB], in_=sel_t[:, 2 * NB : 5 * NB]
                ),
                5: lambda: nc.scalar.dma_start(        # patterns 5-9, block 5
                    out=sel_s[:, 5 * NB :], in_=sel_t[:, 5 * NB :]
                ),
                8: lambda: nc.scalar.dma_start(        # block 12
                    out=npgb_s[2][:], in_=npgb_t[2][:]
                ),
                14: lambda: nc.scalar.dma_start(       # block 20
                    out=qgb_s[:, 2 * H2 :], in_=qgb_t[:, 2 * H2 :]
                ),
                40: lambda: nc.scalar.dma_start(       # block 52
                    out=npgb_s[3][:], in_=npgb_t[3][:]
                ),
            }

            # ---- tensor-engine warm-up ---------------------------------
            # The PE runs at 1.2 GHz until ~3us of continuous execution.
            # The first real matmul waits ~9us for its DMAs; fill that
            # window with matmuls on never-written scratch SBUF (values
            # are garbage, results are discarded) so the clock is at
            # 2.4 GHz when real work arrives.
            wu_w = cp.tile([P, P], BF16, name="wu_w")
            wu_x = cp.tile([P, NB], BF16, name="wu_x")
            nc.vector.memset(wu_w[:], 0.5)
            nc.vector.memset(wu_x[:], 0.5)
            for wi in range(2):
                wu_ps = pp.tile([P, 1024], F32, tag="h1", bufs=2, name=f"wups{wi}")
                for wj in range(8):
                    nc.tensor.matmul(
                        out=wu_ps[:, 0:NB],
                        lhsT=wu_w[:],
                        rhs=wu_x[:],
                        start=(wj == 0),
                        stop=(wj == 7),
                    )

            qgb_v = qgb_s.rearrange("p (t m) -> p t m", m=H2)

            # ---- main loop: software pipeline over blocks ---------------
            # stage A(b): xt load + L1 matmuls + fused max(a, -pgb) on DVE
            # stage B(b): L2 matmuls + ACT copy + GpSimd max  (iter b+3)
            # stage C(b): L3 quad matmul + quad output        (iter b+6)
            st = {}
            xt_tiles = {}

            def stage_a(b):
                pr, half = divmod(b, 2)
                if half == 0:
                    xt_s = wp.tile([P, 2 * NB], BF16, tag="xt", bufs=4, name=f"xt{pr}")
                    if pr == 0:  # split the first load so block 0 starts early
                        nc.sync.dma_start(out=xt_s[:, 0:NB], in_=xt_t[0, :, 0:NB])
                        nc.sync.dma_start(
                            out=xt_s[:, NB : 2 * NB], in_=xt_t[0, :, NB : 2 * NB]
                        )
                    else:
                        nc.sync.dma_start(out=xt_s[:], in_=xt_t[pr])
                    xt_tiles[pr] = xt_s
                xin = xt_tiles[b // 2][:, half * NB : (half + 1) * NB]
                h1ps = pp.tile([P, 1024], F32, tag="h1", bufs=2, name=f"h1ps{b}")
                for c in range(2):
                    nc.tensor.matmul(
                        out=h1ps[:, c * 512 : c * 512 + NB],
                        lhsT=w1a_s[:, c * P : (c + 1) * P],
                        rhs=xin,
                        start=True,
                        stop=True,
                    )
                # h1' = max(a, -pgb[g]) with -pgb broadcast over each
                # graph's 40 nodes (stride-0 inner dim) on the DVE
                ck, goff = _chunk_of(b)
                npgb_v = npgb_s[ck].rearrange("p (c g) -> p c g", c=2)
                h1s = hp.tile([P, 2 * NB], BF16, tag="h1s", bufs=6, name=f"h1s{b}")
                for c in range(2):
                    bias = (
                        npgb_v[:, c, goff : goff + GPB]
                        .unsqueeze(2)
                        .broadcast_to([P, GPB, NPG])
                    )
                    nc.vector.scalar_tensor_tensor(
                        out=h1s[:, c * NB : (c + 1) * NB].rearrange(
                            "p (g n) -> p g n", n=NPG
                        ),
                        in0=h1ps[:, c * 512 : c * 512 + NB].rearrange(
                            "p (g n) -> p g n", n=NPG
                        ),
                        scalar=1.0,
                        in1=bias,
                        op0=MULT,
                        op1=MAX,
                    )
                st[b] = {"h1s": h1s}

            def stage_b(b):
                h1s = st[b]["h1s"]
                routed = _is_routed(b)
                h2ps = pp.tile([P, NB], F32, tag="h2", bufs=3, name=f"h2ps{b}")
                if not routed:
                    # per-graph bias qgb (with b2 folded in) via one one-hot
                    # selector matmul; blocks never straddle qgb tiles
                    t, m = divmod(b, BPT)
                    nc.tensor.matmul(
                        out=h2ps[:],
                        lhsT=qgb_v[:, t, :],
                        rhs=sel_s[:, m * NB : (m + 1) * NB],
                        start=True,
                        stop=False,
                    )
                nc.tensor.matmul(
                    out=h2ps[:], lhsT=w2a_s[:], rhs=h1s[:, 0:NB],
                    start=routed, stop=False,
                )
                nc.tensor.matmul(
                    out=h2ps[:], lhsT=w2b_s[:], rhs=h1s[:, NB : 2 * NB],
                    start=False, stop=True,
                )
                h2s = hp.tile([P, NB], BF16, tag="h2s", bufs=8, name=f"h2s{b}")
                if routed:
                    # h2' = max(h2ps, -qgb[g]) on the DVE; the dropped
                    # +qgb[g] term resurfaces as qgb@W3 added on the host
                    r = ROUTED_IDX[b]
                    qbias = (
                        nqgb_s[:, r * GPB : (r + 1) * GPB]
                        .unsqueeze(2)
                        .broadcast_to([P, GPB, NPG])
                    )
                    nc.vector.scalar_tensor_tensor(
                        out=h2s.rearrange("p (g n) -> p g n", n=NPG),
                        in0=h2ps.rearrange("p (g n) -> p g n", n=NPG),
                        scalar=1.0,
                        in1=qbias,
                        op0=MULT,
                        op1=MAX,
                    )
                else:
                    nc.scalar.activation(out=h2s[:], in_=h2ps[:], func=RELU)
                st[b]["h2s"] = h2s

            def stage_c(b):
                # emit the whole quad's L3 matmuls together so the masked
                # (tile_position) LDWEIGHTS bubbles cluster once per quad
                q, p4 = divmod(b, 4)
                if p4 != 3:
                    return
                l3ps = pp.tile([P, NB], F32, tag="l3", bufs=1, name=f"l3ps{q}")
                for p in range(4):
                    nc.tensor.matmul(
                        out=l3ps[32 * p : 32 * p + 32, :],
                        lhsT=w3_s[:],
                        rhs=st[4 * q + p]["h2s"][:],
                        start=True,
                        stop=True,
                        skip_group_check=True,
                        tile_position=(0, 32 * p),
                    )
                oq = hp.tile([P, NB], F32, tag="oq", bufs=2, name=f"oq{q}")
                nc.scalar.activation(
                    out=oq[0:97, :], in_=l3ps[0:97, :],
                    func=IDENT, bias=b3_s[0:97, 0:1],
                )
                oq4 = oq.rearrange("(a b) n -> a b n", b=32)[:, 0, :]
                nc.sync.dma_start(out=out_t[q], in_=oq4)
                for p in range(4):
                    del st[4 * q + p]

            for b in range(BLOCKS + 6):
                if b in deferred:
                    deferred[b]()
                if b < BLOCKS:
                    stage_a(b)
                if 0 <= b - 3 < BLOCKS:
                    stage_b(b - 3)
                if 0 <= b - 6 < BLOCKS:
                    stage_c(b - 6)

    return nc


def _get_program():
    global _PROGRAM
    if _PROGRAM is None:
        _PROGRAM = _build_program()
        _PROGRAM.finalize()  # Bacc: wait-splitting + reg alloc passes
    return _PROGRAM


def _uniform_structure(node_to_graphid, graph_offsets):
    n2g = np.asarray(node_to_graphid)
    go = np.asarray(graph_offsets)
    if n2g.shape != (N_NODES,) or go.shape != (N_GRAPHS,):
        return False
    if not np.array_equal(go, np.arange(N_GRAPHS, dtype=go.dtype) * NPG):
        return False
    expect = np.repeat(np.arange(N_GRAPHS, dtype=n2g.dtype), NPG)
    return np.array_equal(n2g, expect)


def _reference_numpy(node_features, prev_action_per_graph, context_vectors_per_graph,
                     node_to_graphid, graph_offsets, W1, b1, W2, b2, W3, b3):
    prev_abs = np.asarray(graph_offsets) + np.asarray(prev_action_per_graph)
    prev_per_node = node_features[prev_abs][node_to_graphid]
    ctx_per_node = context_vectors_per_graph[node_to_graphid]
    x = np.concatenate([node_features, prev_per_node, ctx_per_node], axis=1)
    h = np.maximum(x @ W1 + b1, 0.0)
    h = np.maximum(h @ W2 + b2, 0.0)
    return (h @ W3 + b3).astype(np.float32)


def make_in_maps(inputs):
    """Host-side shard + layout prep.  Returns (in_maps, counts, rgb_all)."""
    nf = np.ascontiguousarray(np.asarray(inputs["node_features"], dtype=np.float32))
    ctx = np.ascontiguousarray(
        np.asarray(inputs["context_vectors_per_graph"], dtype=np.float32)
    )
    W1 = np.asarray(inputs["W1"], dtype=np.float32)
    b1 = np.asarray(inputs["b1"], dtype=np.float32)
    W2 = np.asarray(inputs["W2"], dtype=np.float32)
    b2 = np.asarray(inputs["b2"], dtype=np.float32)
    W3 = np.asarray(inputs["W3"], dtype=np.float32)
    b3 = np.asarray(inputs["b3"], dtype=np.float32)

    prev_abs = (
        np.asarray(inputs["graph_offsets"]).astype(np.int64)
        + np.asarray(inputs["prev_action_per_graph"]).astype(np.int64)
    )
    # per-graph biases, computed on the host (tiny GEMMs)
    pgb_all = nf[prev_abs] @ W1[D : 2 * D] + ctx @ W1[2 * D :] + b1  # [G, 256]
    qgb_all = pgb_all @ W2 + b2                                      # [G, 128]
    rgb_all = (qgb_all @ W3).reshape(-1)                             # [G]

    # graph shard boundaries: 4 cores x 1563 + 4 cores x 1562
    base, rem = divmod(N_GRAPHS, N_CORES)
    counts = [base + (1 if c < rem else 0) for c in range(N_CORES)]
    bounds = np.concatenate([[0], np.cumsum(counts)])

    # shared constants (matmul operands as bf16)
    w1a = np.ascontiguousarray(W1[0:D]).astype(BF16_NP)
    w2bf = np.ascontiguousarray(W2).astype(BF16_NP)
    w3 = np.ascontiguousarray(np.repeat(W3.reshape(H2, 1), 32, axis=1)).astype(BF16_NP)
    b3r = np.full((P, 1), float(np.asarray(b3).reshape(-1)[0]), dtype=np.float32)

    # one-hot selector patterns: pattern m maps qgb-tile partition 12m+j
    # to columns [40j, 40j+40)
    sel = np.zeros((P, NPAT, NB), dtype=np.float32)
    for m_ in range(NPAT):
        for j in range(GPB):
            sel[GPB * m_ + j, m_, j * NPG : (j + 1) * NPG] = 1.0
    sel_bf = np.ascontiguousarray(sel.reshape(P, NPAT * NB)).astype(BF16_NP)

    in_maps = []
    for c in range(N_CORES):
        gs, ge = int(bounds[c]), int(bounds[c + 1])
        gcount = ge - gs
        ns, ne = NPG * gs, NPG * ge

        nf_c = np.zeros((NODES_PC, D), dtype=np.float32)
        nf_c[: ne - ns] = nf[ns:ne]
        xt_c = np.ascontiguousarray(
            nf_c.reshape(PAIRS, 2, NB, D).transpose(0, 3, 1, 2).reshape(PAIRS, D, 2 * NB)
        ).astype(BF16_NP)

        npgb_pad = np.zeros((G_PC, H1), dtype=np.float32)
        npgb_pad[:gcount] = -pgb_all[gs:ge]
        npgb_bf = npgb_pad.astype(BF16_NP)
        npgb_tiles = []
        off = 0
        for g in NCH_G:
            # [P, 2, g] with layout (h1dim%128, chunk, graph)
            blk = npgb_bf[off : off + g]                  # [g, 256]
            blk = blk.T.reshape(2, P, g).transpose(1, 0, 2)
            npgb_tiles.append(np.ascontiguousarray(blk.reshape(P, 2 * g)))
            off += g

        # qgb tiles: [NQT, P, H2] with 120 graphs per 128-partition tile
        qgb_arr = np.zeros((NQT, P, H2), dtype=np.float32)
        qcore = qgb_all[gs:ge]
        for t in range(NQT):
            lo = t * GPT
            hi = min(lo + GPT, gcount)
            if lo < gcount:
                qgb_arr[t, : hi - lo] = qcore[lo:hi]
        qgb_c = np.ascontiguousarray(
            qgb_arr.astype(BF16_NP).transpose(1, 0, 2).reshape(P, NQT * H2)
        )

        # -qgb for the DVE-routed blocks, [h2dim, routed-graph] layout
        nqgb_arr = np.zeros((N_ROUTED * GPB, H2), dtype=np.float32)
        for r, b in enumerate(ROUTED):
            lo = b * GPB
            hi = min(lo + GPB, gcount)
            if lo < gcount:
                nqgb_arr[r * GPB : r * GPB + hi - lo] = -qcore[lo:hi]
        nqgb_c = np.ascontiguousarray(nqgb_arr.astype(BF16_NP).T)

        m = {
            "xt": xt_c,
            "qgb": qgb_c,
            "nqgb": nqgb_c,
            "sel": sel_bf,
            "w1a": w1a,
            "w2": w2bf,
            "w3": w3,
            "b3": b3r,
        }
        for k, t in enumerate(npgb_tiles):
            m[f"npgb{k}"] = t
        in_maps.append(m)

    # host-side correction vector: routed graphs get their dropped
    # qgb@W3 term back (b3 is applied on-device for all nodes)
    corr = np.zeros(N_GRAPHS, dtype=np.float32)
    for c in range(N_CORES):
        gs, ge = int(bounds[c]), int(bounds[c + 1])
        local = np.arange(ge - gs)
        routed_set = np.zeros(BLOCKS, dtype=bool)
        routed_set[ROUTED] = True
        corr[gs:ge] = np.where(routed_set[local // GPB], rgb_all[gs:ge], 0.0)
    return in_maps, counts, corr


LAST_RESULTS = None  # BassKernelResults of the most recent kernel() call


def kernel(**inputs) -> np.ndarray:
    global LAST_RESULTS
    if not _uniform_structure(inputs["node_to_graphid"], inputs["graph_offsets"]):
        # Structure differs from the oracle's fixed layout (40 nodes/graph,
        # offsets = 40*g); fall back to a straight host computation.
        return _reference_numpy(**inputs)

    in_maps, counts, corr = make_in_maps(inputs)
    nc = _get_program()
    res = run_bass_kernel_spmd(nc, in_maps, core_ids=list(range(N_CORES)))
    LAST_RESULTS = res
    pieces = []
    for c in range(N_CORES):
        flat = res.results[c]["out"].reshape(-1)
        pieces.append(flat[: NPG * counts[c]])
    full = np.concatenate(pieces) + np.repeat(corr, NPG)
    return full.reshape(N_NODES, 1).astype(np.float32)


if __name__ == "__main__":
    # smoke-trace the program without running it
    prog = _get_program()
    print("traced OK:", len(prog.m.functions[0].instructions)
          if hasattr(prog.m.functions[0], "instructions") else "n/a")


# revision 48
# speedup vs baseline: 1.0050x; 1.0050x over previous
"""Trainium2 Bass kernel for the ActionSelector GNN-MLP problem.

Model (per node n, graph g = graph of n):
    x      = [node_feat(n) | node_feat(prev(g)) | ctx(g)]   # 320
    h1     = relu(x @ W1 + b1)                              # 256
    h2     = relu(h1 @ W2 + b2)                             # 128
    logits = h2 @ W3 + b3                                   # 1

Strategy: data-parallel over graphs across 8 cores.  Per core the MLP is
decomposed with the per-graph pieces precomputed on the host:
    pgb[g] = prev_feat[g] @ W1b + ctx[g] @ W1c + b1         # [G, 256]
    qgb[g] = pgb[g] @ W2 + b2                               # [G, 128]
and on device (exact algebra, no approximation):
    a   = node_feat @ W1a                 (2 bf16 matmuls)
    h1' = max(a, -pgb[g])                 (= relu(a+pgb) - pgb; fused DVE max
                                           with a stride-0 broadcast operand)
    h2  = relu(h1' @ W2 + qgb[g])         (qgb broadcast via ONE one-hot
                                           selector matmul in h2 space)
    out = h2 @ W3 + b3
qgb lives in 120-graph tiles (10 blocks each) so a block's 12 graphs never
straddle a tile boundary: exactly one selector matmul per block, from a
fixed set of 10 one-hot patterns.  Nodes are contiguous by graph
(40/graph, blocks of 12 graphs).  npgb is chunked progressively so the
first blocks only wait on tiny const DMAs.
"""

import os
import sys

import ml_dtypes
import numpy as np

BF16_NP = ml_dtypes.bfloat16

try:
    import concourse.bass as bass  # noqa: F401
except ImportError:  # harness containers keep the repo here
    sys.path.insert(0, "/opt/trn_rl_repo")

import concourse.bacc as bacc
import concourse.bass as bass
import concourse.mybir as mybir
import concourse.tile as tile
from concourse.bass_utils import run_bass_kernel_spmd

F32 = mybir.dt.float32
BF16 = mybir.dt.bfloat16

P = 128
D = 128          # node feature dim
DCTX = 64
H1 = 256
H2 = 128
NPG = 40         # nodes per graph
N_GRAPHS = 12500
N_NODES = N_GRAPHS * NPG

N_CORES = 8
GPB = 12                   # graphs per block
NB = GPB * NPG             # 480 nodes per block
BLOCKS = 132               # blocks per core
QUADS = BLOCKS // 4
G_PC = BLOCKS * GPB        # 1584 graphs per core (padded)
NODES_PC = BLOCKS * NB     # 63360 nodes per core (padded)
PAIRS = BLOCKS // 2

# npgb (broadcast operand) chunking: progressive sizes so the first
# blocks only wait on tiny DMAs
NCH_BLOCKS = [2, 10, 40, 80]
NCH_G = [b * GPB for b in NCH_BLOCKS]   # graphs per chunk
NCH = len(NCH_BLOCKS)

# qgb tiling: 120 graphs (10 blocks) per 128-partition tile so a block's
# 12 graphs never straddle tiles; block b -> tile b//10, pattern b%10
BPT = 10                       # blocks per qgb tile
GPT = BPT * GPB                # 120 graphs per tile
NQT = (BLOCKS + BPT - 1) // BPT  # 14 qgb tiles
NPAT = BPT                     # 10 selector patterns

# mixed bias routing: every ROUTE_EVERY-th block takes its L2 bias as a
# DVE max (h2' = max(h2ps, -qgb), per-graph qgb@W3 re-added on the host)
# instead of the one-hot selector matmul, trading idle DVE capacity for
# tensor-engine cycles
ROUTE_EVERY = 4


def _is_routed(b):
    # every 4th block: keeps the DVE under the PE rate in EVERY 4-block
    # window (the previous denser routing made DVE bursts that stalled
    # the PE ~907ns every ~5 blocks on the h1s dependency)
    return b % ROUTE_EVERY == ROUTE_EVERY - 1


ROUTED = [b for b in range(BLOCKS) if _is_routed(b)]
N_ROUTED = len(ROUTED)
ROUTED_IDX = {b: r for r, b in enumerate(ROUTED)}

_PROGRAM = None


def _chunk_of(b):
    """(chunk index, local graph offset) for block b."""
    acc = 0
    for k, nb in enumerate(NCH_BLOCKS):
        if b < acc + nb:
            return k, GPB * (b - acc)
        acc += nb
    raise AssertionError(b)


def _build_program():
    nc = bacc.Bacc(None, target_bir_lowering=False, debug=False)

    xt_t = nc.dram_tensor("xt", [PAIRS, P, 2 * NB], BF16, kind="ExternalInput")
    npgb_t = [
        nc.dram_tensor(f"npgb{k}", [P, 2 * g], BF16, kind="ExternalInput")
        for k, g in enumerate(NCH_G)
    ]
    qgb_t = nc.dram_tensor("qgb", [P, NQT * H2], BF16, kind="ExternalInput")
    nqgb_t = nc.dram_tensor("nqgb", [P, N_ROUTED * GPB], BF16, kind="ExternalInput")
    sel_t = nc.dram_tensor("sel", [P, NPAT * NB], BF16, kind="ExternalInput")
    w1a_t = nc.dram_tensor("w1a", [D, H1], BF16, kind="ExternalInput")
    w2_t = nc.dram_tensor("w2", [H1, H2], BF16, kind="ExternalInput")
    w3_t = nc.dram_tensor("w3", [H2, 32], BF16, kind="ExternalInput")
    b3_t = nc.dram_tensor("b3", [P, 1], F32, kind="ExternalInput")
    out_t = nc.dram_tensor("out", [QUADS, 4, NB], F32, kind="ExternalOutput")

    RELU = mybir.ActivationFunctionType.Relu
    IDENT = mybir.ActivationFunctionType.Identity
    MULT = mybir.AluOpType.mult
    MAX = mybir.AluOpType.max

    with tile.TileContext(nc) as tc:
        with (
            tc.tile_pool(name="const", bufs=1) as cp,
            tc.tile_pool(name="work", bufs=3) as wp,
            tc.tile_pool(name="hbuf", bufs=4) as hp,
            tc.tile_pool(name="psum", bufs=2, space="PSUM") as pp,
        ):
            # ---- resident constants -------------------------------------
            w1a_s = cp.tile([D, H1], BF16)
            w2a_s = cp.tile([P, H2], BF16)
            w2b_s = cp.tile([P, H2], BF16)
            w3_s = cp.tile([H2, 32], BF16)
            b3_s = cp.tile([P, 1], F32)
            qgb_s = cp.tile([P, NQT * H2], BF16)
            nqgb_s = cp.tile([P, N_ROUTED * GPB], BF16)
            sel_s = cp.tile([P, NPAT * NB], BF16)
            npgb_s = [
                cp.tile([P, 2 * g], BF16, name=f"npgb_s{k}")
                for k, g in enumerate(NCH_G)
            ]

            # earliest-needed first.  The gpsimd queue is deliberately
            # UNUSED for DMA: its dma path is SWDGE (Q7 software descriptor
            # generation, ~1us fixed overhead per dma and a ~4us drain in
            # the epilogue).  sync carries xt + output; consts are split
            # between the scalar and vector HWDGE queues.
            nc.scalar.dma_start(out=w1a_s[:], in_=w1a_t[:])          # block 0
            nc.scalar.dma_start(out=npgb_s[0][:], in_=npgb_t[0][:])  # block 0
            nc.scalar.dma_start(out=npgb_s[1][:], in_=npgb_t[1][:])  # block 2
            nc.scalar.dma_start(out=w2a_s[:], in_=w2_t[0:P, :])      # iter 3
            nc.scalar.dma_start(out=w2b_s[:], in_=w2_t[P : 2 * P, :])
            nc.scalar.dma_start(                                     # iter 3
                out=sel_s[:, 0 : 2 * NB], in_=sel_t[:, 0 : 2 * NB]
            )
            nc.scalar.dma_start(                                     # iter 3
                out=qgb_s[:, 0 : 2 * H2], in_=qgb_t[:, 0 : 2 * H2]
            )
            nc.scalar.dma_start(out=w3_s[:], in_=w3_t[:])            # iter 6
            nc.scalar.dma_start(out=b3_s[:], in_=b3_t[:])

            # remaining big consts are dispatched inside the loop (below)
            # so they do not contend with the first xt tiles for DMA
            # bandwidth; keyed by iteration, comfortably before first use
            deferred = {
                1: lambda: nc.scalar.dma_start(        # iter 8
                    out=nqgb_s[:], in_=nqgb_t[:]
                ),
                2: lambda: nc.scalar.dma_start(        # patterns 2-4, block 2
                    out=sel_s[:, 2 * NB : 5 * NB], in_=sel_t[:, 2 * NB : 5 * NB]
                ),
                5: lambda: nc.scalar.dma_start(        # patterns 5-9, block 5
                    out=sel_s[:, 5 * NB :], in_=sel_t[:, 5 * NB :]
                ),
                8: lambda: nc.scalar.dma_start(        # block 12
                    out=npgb_s[2][:], in_=npgb_t[2][:]
                ),
                14: lambda: nc.scalar.dma_start(       # block 20
                    out=qgb_s[:, 2 * H2 :], in_=qgb_t[:, 2 * H2 :]
                ),
                40: lambda: nc.scalar.dma_start(       # block 52
                    out=npgb_s[3][:], in_=npgb_t[3][:]
                ),
            }

            # ---- tensor-engine warm-up ---------------------------------
            # The PE runs at 1.2 GHz until ~3us of continuous execution.
            # The first real matmul waits ~9us for its DMAs; fill that
            # window with matmuls on never-written scratch SBUF (values
            # are garbage, results are discarded) so the clock is at
            # 2.4 GHz when real work arrives.
            wu_w = cp.tile([P, P], BF16, name="wu_w")
            wu_x = cp.tile([P, NB], BF16, name="wu_x")
            nc.vector.memset(wu_w[:], 0.5)
            nc.vector.memset(wu_x[:], 0.5)
            for wi in range(2):
                wu_ps = pp.tile([P, 1024], F32, tag="h1", bufs=2, name=f"wups{wi}")
                for wj in range(8):
                    nc.tensor.matmul(
                        out=wu_ps[:, 0:NB],
                        lhsT=wu_w[:],
                        rhs=wu_x[:],
                        start=(wj == 0),
                        stop=(wj == 7),
                    )

            qgb_v = qgb_s.rearrange("p (t m) -> p t m", m=H2)

            # ---- main loop: software pipeline over blocks ---------------
            # stage A(b): xt load + L1 matmuls + fused max(a, -pgb) on DVE
            # stage B(b): L2 matmuls + ACT copy + GpSimd max  (iter b+3)
            # stage C(b): L3 quad matmul + quad output        (iter b+6)
            st = {}
            xt_tiles = {}

            def stage_a(b):
                pr, half = divmod(b, 2)
                if half == 0:
                    xt_s = wp.tile([P, 2 * NB], BF16, tag="xt", bufs=4, name=f"xt{pr}")
                    if pr == 0:  # split the first load so block 0 starts early
                        nc.sync.dma_start(out=xt_s[:, 0:NB], in_=xt_t[0, :, 0:NB])
                        nc.sync.dma_start(
                            out=xt_s[:, NB : 2 * NB], in_=xt_t[0, :, NB : 2 * NB]
                        )
                    else:
                        nc.sync.dma_start(out=xt_s[:], in_=xt_t[pr])
                    xt_tiles[pr] = xt_s
                xin = xt_tiles[b // 2][:, half * NB : (half + 1) * NB]
                h1ps = pp.tile([P, 1024], F32, tag="h1", bufs=2, name=f"h1ps{b}")
                for c in range(2):
                    nc.tensor.matmul(
                        out=h1ps[:, c * 512 : c * 512 + NB],
                        lhsT=w1a_s[:, c * P : (c + 1) * P],
                        rhs=xin,
                        start=True,
                        stop=True,
                    )
                # h1' = max(a, -pgb[g]) with -pgb broadcast over each
                # graph's 40 nodes (stride-0 inner dim) on the DVE
                ck, goff = _chunk_of(b)
                npgb_v = npgb_s[ck].rearrange("p (c g) -> p c g", c=2)
                h1s = hp.tile([P, 2 * NB], BF16, tag="h1s", bufs=6, name=f"h1s{b}")
                for c in range(2):
                    bias = (
                        npgb_v[:, c, goff : goff + GPB]
                        .unsqueeze(2)
                        .broadcast_to([P, GPB, NPG])
                    )
                    nc.vector.scalar_tensor_tensor(
                        out=h1s[:, c * NB : (c + 1) * NB].rearrange(
                            "p (g n) -> p g n", n=NPG
                        ),
                        in0=h1ps[:, c * 512 : c * 512 + NB].rearrange(
                            "p (g n) -> p g n", n=NPG
                        ),
                        scalar=1.0,
                        in1=bias,
                        op0=MULT,
                        op1=MAX,
                    )
                st[b] = {"h1s": h1s}

            def stage_b(b):
                h1s = st[b]["h1s"]
                routed = _is_routed(b)
                h2ps = pp.tile([P, NB], F32, tag="h2", bufs=3, name=f"h2ps{b}")
                if not routed:
                    # per-graph bias qgb (with b2 folded in) via one one-hot
                    # selector matmul; blocks never straddle qgb tiles
                    t, m = divmod(b, BPT)
                    nc.tensor.matmul(
                        out=h2ps[:],
                        lhsT=qgb_v[:, t, :],
                        rhs=sel_s[:, m * NB : (m + 1) * NB],
                        start=True,
                        stop=False,
                    )
                nc.tensor.matmul(
                    out=h2ps[:], lhsT=w2a_s[:], rhs=h1s[:, 0:NB],
                    start=routed, stop=False,
                )
                nc.tensor.matmul(
                    out=h2ps[:], lhsT=w2b_s[:], rhs=h1s[:, NB : 2 * NB],
                    start=False, stop=True,
                )
                h2s = hp.tile([P, NB], BF16, tag="h2s", bufs=8, name=f"h2s{b}")
                if routed:
                    # h2' = max(h2ps, -qgb[g]) on the DVE; the dropped
                    # +qgb[g] term resurfaces as qgb@W3 added on the host
                    r = ROUTED_IDX[b]
                    qbias = (
                        nqgb_s[:, r * GPB : (r + 1) * GPB]
                        .unsqueeze(2)
                        .broadcast_to([P, GPB, NPG])
                    )
                    nc.vector.scalar_tensor_tensor(
                        out=h2s.rearrange("p (g n) -> p g n", n=NPG),
                        in0=h2ps.rearrange("p (g n) -> p g n", n=NPG),
                        scalar=1.0,
                        in1=qbias,
                        op0=MULT,
                        op1=MAX,
                    )
                else:
                    nc.scalar.activation(out=h2s[:], in_=h2ps[:], func=RELU)
                st[b]["h2s"] = h2s

            def stage_c(b):
                # emit the whole quad's L3 matmuls together so the masked
                # (tile_position) LDWEIGHTS bubbles cluster once per quad
                q, p4 = divmod(b, 4)
                if p4 != 3:
                    return
                l3ps = pp.tile([P, NB], F32, tag="l3", bufs=1, name=f"l3ps{q}")
                for p in range(4):
                    nc.tensor.matmul(
                        out=l3ps[32 * p : 32 * p + 32, :],
                        lhsT=w3_s[:],
                        rhs=st[4 * q + p]["h2s"][:],
                        start=True,
                        stop=True,
                        skip_group_check=True,
                        tile_position=(0, 32 * p),
                    )
                oq = hp.tile([P, NB], F32, tag="oq", bufs=2, name=f"oq{q}")
                nc.scalar.activation(
                    out=oq[0:97, :], in_=l3ps[0:97, :],
                    func=IDENT, bias=b3_s[0:97, 0:1],
                )
                oq4 = oq.rearrange("(a b) n -> a b n", b=32)[:, 0, :]
                nc.sync.dma_start(out=out_t[q], in_=oq4)
                for p in range(4):
                    del st[4 * q + p]

            for b in range(BLOCKS + 6):
                if b in deferred:
                    deferred[b]()
                if b < BLOCKS:
                    stage_a(b)
                if 0 <= b - 4 < BLOCKS:
                    stage_b(b - 4)
                if 0 <= b - 6 < BLOCKS:
                    stage_c(b - 6)

    return nc


def _get_program():
    global _PROGRAM
    if _PROGRAM is None:
        _PROGRAM = _build_program()
        _PROGRAM.finalize()  # Bacc: wait-splitting + reg alloc passes
    return _PROGRAM


def _uniform_structure(node_to_graphid, graph_offsets):
    n2g = np.asarray(node_to_graphid)
    go = np.asarray(graph_offsets)
    if n2g.shape != (N_NODES,) or go.shape != (N_GRAPHS,):
        return False
    if not np.array_equal(go, np.arange(N_GRAPHS, dtype=go.dtype) * NPG):
        return False
    expect = np.repeat(np.arange(N_GRAPHS, dtype=n2g.dtype), NPG)
    return np.array_equal(n2g, expect)


def _reference_numpy(node_features, prev_action_per_graph, context_vectors_per_graph,
                     node_to_graphid, graph_offsets, W1, b1, W2, b2, W3, b3):
    prev_abs = np.asarray(graph_offsets) + np.asarray(prev_action_per_graph)
    prev_per_node = node_features[prev_abs][node_to_graphid]
    ctx_per_node = context_vectors_per_graph[node_to_graphid]
    x = np.concatenate([node_features, prev_per_node, ctx_per_node], axis=1)
    h = np.maximum(x @ W1 + b1, 0.0)
    h = np.maximum(h @ W2 + b2, 0.0)
    return (h @ W3 + b3).astype(np.float32)


def make_in_maps(inputs):
    """Host-side shard + layout prep.  Returns (in_maps, counts, rgb_all)."""
    nf = np.ascontiguousarray(np.asarray(inputs["node_features"], dtype=np.float32))
    ctx = np.ascontiguousarray(
        np.asarray(inputs["context_vectors_per_graph"], dtype=np.float32)
    )
    W1 = np.asarray(inputs["W1"], dtype=np.float32)
    b1 = np.asarray(inputs["b1"], dtype=np.float32)
    W2 = np.asarray(inputs["W2"], dtype=np.float32)
    b2 = np.asarray(inputs["b2"], dtype=np.float32)
    W3 = np.asarray(inputs["W3"], dtype=np.float32)
    b3 = np.asarray(inputs["b3"], dtype=np.float32)

    prev_abs = (
        np.asarray(inputs["graph_offsets"]).astype(np.int64)
        + np.asarray(inputs["prev_action_per_graph"]).astype(np.int64)
    )
    # per-graph biases, computed on the host (tiny GEMMs)
    pgb_all = nf[prev_abs] @ W1[D : 2 * D] + ctx @ W1[2 * D :] + b1  # [G, 256]
    qgb_all = pgb_all @ W2 + b2                                      # [G, 128]
    rgb_all = (qgb_all @ W3).reshape(-1)                             # [G]

    # graph shard boundaries: 4 cores x 1563 + 4 cores x 1562
    base, rem = divmod(N_GRAPHS, N_CORES)
    counts = [base + (1 if c < rem else 0) for c in range(N_CORES)]
    bounds = np.concatenate([[0], np.cumsum(counts)])

    # shared constants (matmul operands as bf16)
    w1a = np.ascontiguousarray(W1[0:D]).astype(BF16_NP)
    w2bf = np.ascontiguousarray(W2).astype(BF16_NP)
    w3 = np.ascontiguousarray(np.repeat(W3.reshape(H2, 1), 32, axis=1)).astype(BF16_NP)
    b3r = np.full((P, 1), float(np.asarray(b3).reshape(-1)[0]), dtype=np.float32)

    # one-hot selector patterns: pattern m maps qgb-tile partition 12m+j
    # to columns [40j, 40j+40)
    sel = np.zeros((P, NPAT, NB), dtype=np.float32)
    for m_ in range(NPAT):
        for j in range(GPB):
            sel[GPB * m_ + j, m_, j * NPG : (j + 1) * NPG] = 1.0
    sel_bf = np.ascontiguousarray(sel.reshape(P, NPAT * NB)).astype(BF16_NP)

    in_maps = []
    for c in range(N_CORES):
        gs, ge = int(bounds[c]), int(bounds[c + 1])
        gcount = ge - gs
        ns, ne = NPG * gs, NPG * ge

        nf_c = np.zeros((NODES_PC, D), dtype=np.float32)
        nf_c[: ne - ns] = nf[ns:ne]
        xt_c = np.ascontiguousarray(
            nf_c.reshape(PAIRS, 2, NB, D).transpose(0, 3, 1, 2).reshape(PAIRS, D, 2 * NB)
        ).astype(BF16_NP)

        npgb_pad = np.zeros((G_PC, H1), dtype=np.float32)
        npgb_pad[:gcount] = -pgb_all[gs:ge]
        npgb_bf = npgb_pad.astype(BF16_NP)
        npgb_tiles = []
        off = 0
        for g in NCH_G:
            # [P, 2, g] with layout (h1dim%128, chunk, graph)
            blk = npgb_bf[off : off + g]                  # [g, 256]
            blk = blk.T.reshape(2, P, g).transpose(1, 0, 2)
            npgb_tiles.append(np.ascontiguousarray(blk.reshape(P, 2 * g)))
            off += g

        # qgb tiles: [NQT, P, H2] with 120 graphs per 128-partition tile
        qgb_arr = np.zeros((NQT, P, H2), dtype=np.float32)
        qcore = qgb_all[gs:ge]
        for t in range(NQT):
            lo = t * GPT
            hi = min(lo + GPT, gcount)
            if lo < gcount:
                qgb_arr[t, : hi - lo] = qcore[lo:hi]
        qgb_c = np.ascontiguousarray(
            qgb_arr.astype(BF16_NP).transpose(1, 0, 2).reshape(P, NQT * H2)
        )

        # -qgb for the DVE-routed blocks, [h2dim, routed-graph] layout
        nqgb_arr = np.zeros((N_ROUTED * GPB, H2), dtype=np.float32)
        for r, b in enumerate(ROUTED):
            lo = b * GPB
            hi = min(lo + GPB, gcount)
            if lo < gcount:
                nqgb_arr[r * GPB : r * GPB + hi - lo] = -qcore[lo:hi]
        nqgb_c = np.ascontiguousarray(nqgb_arr.astype(BF16_NP).T)

        m = {
            "xt": xt_c,
            "qgb": qgb_c,
            "nqgb": nqgb_c,
            "sel": sel_bf,
            "w1a": w1a,
            "w2": w2bf,
            "w3": w3,
            "b3": b3r,
        }
        for k, t in enumerate(npgb_tiles):
            m[f"npgb{k}"] = t
        in_maps.append(m)

    # host-side correction vector: routed graphs get their dropped
    # qgb@W3 term back (b3 is applied on-device for all nodes)
    corr = np.zeros(N_GRAPHS, dtype=np.float32)
    for c in range(N_CORES):
        gs, ge = int(bounds[c]), int(bounds[c + 1])
        local = np.arange(ge - gs)
        routed_set = np.zeros(BLOCKS, dtype=bool)
        routed_set[ROUTED] = True
        corr[gs:ge] = np.where(routed_set[local // GPB], rgb_all[gs:ge], 0.0)
    return in_maps, counts, corr


LAST_RESULTS = None  # BassKernelResults of the most recent kernel() call


def kernel(**inputs) -> np.ndarray:
    global LAST_RESULTS
    if not _uniform_structure(inputs["node_to_graphid"], inputs["graph_offsets"]):
        # Structure differs from the oracle's fixed layout (40 nodes/graph,
        # offsets = 40*g); fall back to a straight host computation.
        return _reference_numpy(**inputs)

    in_maps, counts, corr = make_in_maps(inputs)
    nc = _get_program()
    res = run_bass_kernel_spmd(nc, in_maps, core_ids=list(range(N_CORES)))
    LAST_RESULTS = res
    pieces = []
    for c in range(N_CORES):
        flat = res.results[c]["out"].reshape(-1)
        pieces.append(flat[: NPG * counts[c]])
    full = np.concatenate(pieces) + np.repeat(corr, NPG)
    return full.reshape(N_NODES, 1).astype(np.float32)


if __name__ == "__main__":
    # smoke-trace the program without running it
    prog = _get_program()
    print("traced OK:", len(prog.m.functions[0].instructions)
          if hasattr(prog.m.functions[0], "instructions") else "n/a")


# revision 49
# speedup vs baseline: 1.1429x; 1.1372x over previous
"""Trainium2 Bass kernel for the ActionSelector GNN-MLP problem.

Model (per node n, graph g = graph of n):
    x      = [node_feat(n) | node_feat(prev(g)) | ctx(g)]   # 320
    h1     = relu(x @ W1 + b1)                              # 256
    h2     = relu(h1 @ W2 + b2)                             # 128
    logits = h2 @ W3 + b3                                   # 1

Strategy: data-parallel over graphs across 8 cores.  Per core the MLP is
decomposed with the per-graph pieces precomputed on the host:
    pgb[g] = prev_feat[g] @ W1b + ctx[g] @ W1c + b1         # [G, 256]
    qgb[g] = pgb[g] @ W2 + b2                               # [G, 128]
and on device (exact algebra, no approximation):
    a   = node_feat @ W1a                 (2 bf16 matmuls)
    h1' = max(a, -pgb[g])                 (= relu(a+pgb) - pgb; fused DVE max
                                           with a stride-0 broadcast operand)
    h2  = relu(h1' @ W2 + qgb[g])         (qgb broadcast via ONE one-hot
                                           selector matmul in h2 space)
    out = h2 @ W3 + b3
qgb lives in 120-graph tiles (10 blocks each) so a block's 12 graphs never
straddle a tile boundary: exactly one selector matmul per block, from a
fixed set of 10 one-hot patterns.  Nodes are contiguous by graph
(40/graph, blocks of 12 graphs).  npgb is chunked progressively so the
first blocks only wait on tiny const DMAs.
"""

import os
import sys

import ml_dtypes
import numpy as np

BF16_NP = ml_dtypes.bfloat16

try:
    import concourse.bass as bass  # noqa: F401
except ImportError:  # harness containers keep the repo here
    sys.path.insert(0, "/opt/trn_rl_repo")

import concourse.bacc as bacc
import concourse.bass as bass
import concourse.mybir as mybir
import concourse.tile as tile
from concourse.bass_utils import run_bass_kernel_spmd

F32 = mybir.dt.float32
BF16 = mybir.dt.bfloat16

P = 128
D = 128          # node feature dim
DCTX = 64
H1 = 256
H2 = 128
NPG = 40         # nodes per graph
N_GRAPHS = 12500
N_NODES = N_GRAPHS * NPG

N_CORES = 8
GPB = 12                   # graphs per block
NB = GPB * NPG             # 480 nodes per block
BLOCKS = 132               # blocks per core
QUADS = BLOCKS // 4
G_PC = BLOCKS * GPB        # 1584 graphs per core (padded)
NODES_PC = BLOCKS * NB     # 63360 nodes per core (padded)
PAIRS = BLOCKS // 2

# npgb (broadcast operand) chunking: progressive sizes so the first
# blocks only wait on tiny DMAs
NCH_BLOCKS = [2, 10, 40, 80]
NCH_G = [b * GPB for b in NCH_BLOCKS]   # graphs per chunk
NCH = len(NCH_BLOCKS)

# qgb tiling: 120 graphs (10 blocks) per 128-partition tile so a block's
# 12 graphs never straddle tiles; block b -> tile b//10, pattern b%10
BPT = 10                       # blocks per qgb tile
GPT = BPT * GPB                # 120 graphs per tile
NQT = (BLOCKS + BPT - 1) // BPT  # 14 qgb tiles
NPAT = BPT                     # 10 selector patterns

# mixed bias routing: every ROUTE_EVERY-th block takes its L2 bias as a
# DVE max (h2' = max(h2ps, -qgb), per-graph qgb@W3 re-added on the host)
# instead of the one-hot selector matmul, trading idle DVE capacity for
# tensor-engine cycles
ROUTE_EVERY = 4


def _is_routed(b):
    # every 4th block: keeps the DVE under the PE rate in EVERY 4-block
    # window (the previous denser routing made DVE bursts that stalled
    # the PE ~907ns every ~5 blocks on the h1s dependency)
    return b % ROUTE_EVERY == ROUTE_EVERY - 1


ROUTED = [b for b in range(BLOCKS) if _is_routed(b)]
N_ROUTED = len(ROUTED)
ROUTED_IDX = {b: r for r, b in enumerate(ROUTED)}

_PROGRAM = None


def _chunk_of(b):
    """(chunk index, local graph offset) for block b."""
    acc = 0
    for k, nb in enumerate(NCH_BLOCKS):
        if b < acc + nb:
            return k, GPB * (b - acc)
        acc += nb
    raise AssertionError(b)


def _build_program():
    nc = bacc.Bacc(None, target_bir_lowering=False, debug=False)

    xt_t = nc.dram_tensor("xt", [PAIRS, P, 2 * NB], BF16, kind="ExternalInput")
    npgb_t = [
        nc.dram_tensor(f"npgb{k}", [P, 2 * g], BF16, kind="ExternalInput")
        for k, g in enumerate(NCH_G)
    ]
    qgb_t = nc.dram_tensor("qgb", [P, NQT * H2], BF16, kind="ExternalInput")
    nqgb_t = nc.dram_tensor("nqgb", [P, N_ROUTED * GPB], BF16, kind="ExternalInput")
    sel_t = nc.dram_tensor("sel", [P, NPAT * NB], BF16, kind="ExternalInput")
    w1a_t = nc.dram_tensor("w1a", [D, H1], BF16, kind="ExternalInput")
    w2_t = nc.dram_tensor("w2", [H1, H2], BF16, kind="ExternalInput")
    w3_t = nc.dram_tensor("w3", [H2, 32], BF16, kind="ExternalInput")
    b3_t = nc.dram_tensor("b3", [P, 1], F32, kind="ExternalInput")
    out_t = nc.dram_tensor("out", [QUADS, 4, NB], F32, kind="ExternalOutput")

    RELU = mybir.ActivationFunctionType.Relu
    IDENT = mybir.ActivationFunctionType.Identity
    MULT = mybir.AluOpType.mult
    MAX = mybir.AluOpType.max

    with tile.TileContext(nc) as tc:
        with (
            tc.tile_pool(name="const", bufs=1) as cp,
            tc.tile_pool(name="work", bufs=3) as wp,
            tc.tile_pool(name="hbuf", bufs=4) as hp,
            tc.tile_pool(name="psum", bufs=2, space="PSUM") as pp,
        ):
            # ---- resident constants -------------------------------------
            w1a_s = cp.tile([D, H1], BF16)
            w2a_s = cp.tile([P, H2], BF16)
            w2b_s = cp.tile([P, H2], BF16)
            w3_s = cp.tile([H2, 32], BF16)
            b3_s = cp.tile([P, 1], F32)
            qgb_s = cp.tile([P, NQT * H2], BF16)
            nqgb_s = cp.tile([P, N_ROUTED * GPB], BF16)
            sel_s = cp.tile([P, NPAT * NB], BF16)
            npgb_s = [
                cp.tile([P, 2 * g], BF16, name=f"npgb_s{k}")
                for k, g in enumerate(NCH_G)
            ]

            # earliest-needed first.  The gpsimd queue is deliberately
            # UNUSED for DMA: its dma path is SWDGE (Q7 software descriptor
            # generation, ~1us fixed overhead per dma and a ~4us drain in
            # the epilogue).  sync carries xt + output; consts are split
            # between the scalar and vector HWDGE queues.
            nc.scalar.dma_start(out=w1a_s[:], in_=w1a_t[:])          # block 0
            nc.scalar.dma_start(out=npgb_s[0][:], in_=npgb_t[0][:])  # block 0
            nc.scalar.dma_start(out=npgb_s[1][:], in_=npgb_t[1][:])  # block 2
            nc.scalar.dma_start(out=w2a_s[:], in_=w2_t[0:P, :])      # iter 3
            nc.scalar.dma_start(out=w2b_s[:], in_=w2_t[P : 2 * P, :])
            nc.scalar.dma_start(                                     # iter 3
                out=sel_s[:, 0 : 2 * NB], in_=sel_t[:, 0 : 2 * NB]
            )
            nc.scalar.dma_start(                                     # iter 3
                out=qgb_s[:, 0 : 2 * H2], in_=qgb_t[:, 0 : 2 * H2]
            )
            nc.scalar.dma_start(out=w3_s[:], in_=w3_t[:])            # iter 6
            nc.scalar.dma_start(out=b3_s[:], in_=b3_t[:])

            # remaining big consts are dispatched inside the loop (below)
            # so they do not contend with the first xt tiles for DMA
            # bandwidth; keyed by iteration, comfortably before first use
            deferred = {
                1: lambda: nc.scalar.dma_start(        # iter 8
                    out=nqgb_s[:], in_=nqgb_t[:]
                ),
                2: lambda: nc.scalar.dma_start(        # patterns 2-4, block 2
                    out=sel_s[:, 2 * NB : 5 * NB], in_=sel_t[:, 2 * NB : 5 * NB]
                ),
                5: lambda: nc.scalar.dma_start(        # patterns 5-9, block 5
                    out=sel_s[:, 5 * NB :], in_=sel_t[:, 5 * NB :]
                ),
                8: lambda: nc.scalar.dma_start(        # block 12
                    out=npgb_s[2][:], in_=npgb_t[2][:]
                ),
                14: lambda: nc.scalar.dma_start(       # block 20
                    out=qgb_s[:, 2 * H2 :], in_=qgb_t[:, 2 * H2 :]
                ),
                40: lambda: nc.scalar.dma_start(       # block 52
                    out=npgb_s[3][:], in_=npgb_t[3][:]
                ),
            }

            # ---- tensor-engine warm-up ---------------------------------
            # The PE runs at 1.2 GHz until ~3us of continuous execution.
            # The first real matmul waits ~9us for its DMAs; fill that
            # window with matmuls on never-written scratch SBUF (values
            # are garbage, results are discarded) so the clock is at
            # 2.4 GHz when real work arrives.
            wu_w = cp.tile([P, P], BF16, name="wu_w")
            wu_x = cp.tile([P, NB], BF16, name="wu_x")
            nc.vector.memset(wu_w[:], 0.5)
            nc.vector.memset(wu_x[:], 0.5)
            for wi in range(2):
                wu_ps = pp.tile([P, 1024], F32, tag="h1", bufs=3, name=f"wups{wi}")
                for wj in range(8):
                    nc.tensor.matmul(
                        out=wu_ps[:, 0:NB],
                        lhsT=wu_w[:],
                        rhs=wu_x[:],
                        start=(wj == 0),
                        stop=(wj == 7),
                    )

            qgb_v = qgb_s.rearrange("p (t m) -> p t m", m=H2)

            # ---- main loop: software pipeline over blocks ---------------
            # stage A(b): xt load + L1 matmuls + fused max(a, -pgb) on DVE
            # stage B(b): L2 matmuls + ACT copy + GpSimd max  (iter b+3)
            # stage C(b): L3 quad matmul + quad output        (iter b+6)
            st = {}
            xt_tiles = {}

            def stage_a(b):
                pr, half = divmod(b, 2)
                if half == 0:
                    xt_s = wp.tile([P, 2 * NB], BF16, tag="xt", bufs=4, name=f"xt{pr}")
                    if pr == 0:  # split the first load so block 0 starts early
                        nc.sync.dma_start(out=xt_s[:, 0:NB], in_=xt_t[0, :, 0:NB])
                        nc.sync.dma_start(
                            out=xt_s[:, NB : 2 * NB], in_=xt_t[0, :, NB : 2 * NB]
                        )
                    else:
                        nc.sync.dma_start(out=xt_s[:], in_=xt_t[pr])
                    xt_tiles[pr] = xt_s
                xin = xt_tiles[b // 2][:, half * NB : (half + 1) * NB]
                h1ps = pp.tile([P, 1024], F32, tag="h1", bufs=3, name=f"h1ps{b}")
                for c in range(2):
                    nc.tensor.matmul(
                        out=h1ps[:, c * 512 : c * 512 + NB],
                        lhsT=w1a_s[:, c * P : (c + 1) * P],
                        rhs=xin,
                        start=True,
                        stop=True,
                    )
                # h1' = max(a, -pgb[g]) with -pgb broadcast over each
                # graph's 40 nodes (stride-0 inner dim) on the DVE
                ck, goff = _chunk_of(b)
                npgb_v = npgb_s[ck].rearrange("p (c g) -> p c g", c=2)
                h1s = hp.tile([P, 2 * NB], BF16, tag="h1s", bufs=6, name=f"h1s{b}")
                for c in range(2):
                    bias = (
                        npgb_v[:, c, goff : goff + GPB]
                        .unsqueeze(2)
                        .broadcast_to([P, GPB, NPG])
                    )
                    nc.vector.scalar_tensor_tensor(
                        out=h1s[:, c * NB : (c + 1) * NB].rearrange(
                            "p (g n) -> p g n", n=NPG
                        ),
                        in0=h1ps[:, c * 512 : c * 512 + NB].rearrange(
                            "p (g n) -> p g n", n=NPG
                        ),
                        scalar=1.0,
                        in1=bias,
                        op0=MULT,
                        op1=MAX,
                    )
                st[b] = {"h1s": h1s}

            def stage_b(b):
                h1s = st[b]["h1s"]
                routed = _is_routed(b)
                h2ps = pp.tile([P, NB], F32, tag="h2", bufs=2, name=f"h2ps{b}")
                if not routed:
                    # per-graph bias qgb (with b2 folded in) via one one-hot
                    # selector matmul; blocks never straddle qgb tiles
                    t, m = divmod(b, BPT)
                    nc.tensor.matmul(
                        out=h2ps[:],
                        lhsT=qgb_v[:, t, :],
                        rhs=sel_s[:, m * NB : (m + 1) * NB],
                        start=True,
                        stop=False,
                    )
                nc.tensor.matmul(
                    out=h2ps[:], lhsT=w2a_s[:], rhs=h1s[:, 0:NB],
                    start=routed, stop=False,
                )
                nc.tensor.matmul(
                    out=h2ps[:], lhsT=w2b_s[:], rhs=h1s[:, NB : 2 * NB],
                    start=False, stop=True,
                )
                h2s = hp.tile([P, NB], BF16, tag="h2s", bufs=8, name=f"h2s{b}")
                if routed:
                    # h2' = max(h2ps, -qgb[g]) on the DVE; the dropped
                    # +qgb[g] term resurfaces as qgb@W3 added on the host
                    r = ROUTED_IDX[b]
                    qbias = (
                        nqgb_s[:, r * GPB : (r + 1) * GPB]
                        .unsqueeze(2)
                        .broadcast_to([P, GPB, NPG])
                    )
                    nc.vector.scalar_tensor_tensor(
                        out=h2s.rearrange("p (g n) -> p g n", n=NPG),
                        in0=h2ps.rearrange("p (g n) -> p g n", n=NPG),
                        scalar=1.0,
                        in1=qbias,
                        op0=MULT,
                        op1=MAX,
                    )
                else:
                    nc.scalar.activation(out=h2s[:], in_=h2ps[:], func=RELU)
                st[b]["h2s"] = h2s

            def stage_c(b):
                # emit the whole quad's L3 matmuls together so the masked
                # (tile_position) LDWEIGHTS bubbles cluster once per quad
                q, p4 = divmod(b, 4)
                if p4 != 3:
                    return
                l3ps = pp.tile([P, NB], F32, tag="h2", bufs=2, name=f"l3ps{q}")
                for p in range(4):
                    nc.tensor.matmul(
                        out=l3ps[32 * p : 32 * p + 32, :],
                        lhsT=w3_s[:],
                        rhs=st[4 * q + p]["h2s"][:],
                        start=True,
                        stop=True,
                        skip_group_check=True,
                        tile_position=(0, 32 * p),
                    )
                oq = hp.tile([P, NB], F32, tag="oq", bufs=2, name=f"oq{q}")
                nc.scalar.activation(
                    out=oq[0:97, :], in_=l3ps[0:97, :],
                    func=IDENT, bias=b3_s[0:97, 0:1],
                )
                oq4 = oq.rearrange("(a b) n -> a b n", b=32)[:, 0, :]
                nc.sync.dma_start(out=out_t[q], in_=oq4)
                for p in range(4):
                    del st[4 * q + p]

            for b in range(BLOCKS + 6):
                if b in deferred:
                    deferred[b]()
                if b < BLOCKS:
                    stage_a(b)
                if 0 <= b - 4 < BLOCKS:
                    stage_b(b - 4)
                if 0 <= b - 6 < BLOCKS:
                    stage_c(b - 6)

    return nc


def _get_program():
    global _PROGRAM
    if _PROGRAM is None:
        _PROGRAM = _build_program()
        _PROGRAM.finalize()  # Bacc: wait-splitting + reg alloc passes
    return _PROGRAM


def _uniform_structure(node_to_graphid, graph_offsets):
    n2g = np.asarray(node_to_graphid)
    go = np.asarray(graph_offsets)
    if n2g.shape != (N_NODES,) or go.shape != (N_GRAPHS,):
        return False
    if not np.array_equal(go, np.arange(N_GRAPHS, dtype=go.dtype) * NPG):
        return False
    expect = np.repeat(np.arange(N_GRAPHS, dtype=n2g.dtype), NPG)
    return np.array_equal(n2g, expect)


def _reference_numpy(node_features, prev_action_per_graph, context_vectors_per_graph,
                     node_to_graphid, graph_offsets, W1, b1, W2, b2, W3, b3):
    prev_abs = np.asarray(graph_offsets) + np.asarray(prev_action_per_graph)
    prev_per_node = node_features[prev_abs][node_to_graphid]
    ctx_per_node = context_vectors_per_graph[node_to_graphid]
    x = np.concatenate([node_features, prev_per_node, ctx_per_node], axis=1)
    h = np.maximum(x @ W1 + b1, 0.0)
    h = np.maximum(h @ W2 + b2, 0.0)
    return (h @ W3 + b3).astype(np.float32)


def make_in_maps(inputs):
    """Host-side shard + layout prep.  Returns (in_maps, counts, rgb_all)."""
    nf = np.ascontiguousarray(np.asarray(inputs["node_features"], dtype=np.float32))
    ctx = np.ascontiguousarray(
        np.asarray(inputs["context_vectors_per_graph"], dtype=np.float32)
    )
    W1 = np.asarray(inputs["W1"], dtype=np.float32)
    b1 = np.asarray(inputs["b1"], dtype=np.float32)
    W2 = np.asarray(inputs["W2"], dtype=np.float32)
    b2 = np.asarray(inputs["b2"], dtype=np.float32)
    W3 = np.asarray(inputs["W3"], dtype=np.float32)
    b3 = np.asarray(inputs["b3"], dtype=np.float32)

    prev_abs = (
        np.asarray(inputs["graph_offsets"]).astype(np.int64)
        + np.asarray(inputs["prev_action_per_graph"]).astype(np.int64)
    )
    # per-graph biases, computed on the host (tiny GEMMs)
    pgb_all = nf[prev_abs] @ W1[D : 2 * D] + ctx @ W1[2 * D :] + b1  # [G, 256]
    qgb_all = pgb_all @ W2 + b2                                      # [G, 128]
    rgb_all = (qgb_all @ W3).reshape(-1)                             # [G]

    # graph shard boundaries: 4 cores x 1563 + 4 cores x 1562
    base, rem = divmod(N_GRAPHS, N_CORES)
    counts = [base + (1 if c < rem else 0) for c in range(N_CORES)]
    bounds = np.concatenate([[0], np.cumsum(counts)])

    # shared constants (matmul operands as bf16)
    w1a = np.ascontiguousarray(W1[0:D]).astype(BF16_NP)
    w2bf = np.ascontiguousarray(W2).astype(BF16_NP)
    w3 = np.ascontiguousarray(np.repeat(W3.reshape(H2, 1), 32, axis=1)).astype(BF16_NP)
    b3r = np.full((P, 1), float(np.asarray(b3).reshape(-1)[0]), dtype=np.float32)

    # one-hot selector patterns: pattern m maps qgb-tile partition 12m+j
    # to columns [40j, 40j+40)
    sel = np.zeros((P, NPAT, NB), dtype=np.float32)
    for m_ in range(NPAT):
        for j in range(GPB):
            sel[GPB * m_ + j, m_, j * NPG : (j + 1) * NPG] = 1.0
    sel_bf = np.ascontiguousarray(sel.reshape(P, NPAT * NB)).astype(BF16_NP)

    in_maps = []
    for c in range(N_CORES):
        gs, ge = int(bounds[c]), int(bounds[c + 1])
        gcount = ge - gs
        ns, ne = NPG * gs, NPG * ge

        nf_c = np.zeros((NODES_PC, D), dtype=np.float32)
        nf_c[: ne - ns] = nf[ns:ne]
        xt_c = np.ascontiguousarray(
            nf_c.reshape(PAIRS, 2, NB, D).transpose(0, 3, 1, 2).reshape(PAIRS, D, 2 * NB)
        ).astype(BF16_NP)

        npgb_pad = np.zeros((G_PC, H1), dtype=np.float32)
        npgb_pad[:gcount] = -pgb_all[gs:ge]
        npgb_bf = npgb_pad.astype(BF16_NP)
        npgb_tiles = []
        off = 0
        for g in NCH_G:
            # [P, 2, g] with layout (h1dim%128, chunk, graph)
            blk = npgb_bf[off : off + g]                  # [g, 256]
            blk = blk.T.reshape(2, P, g).transpose(1, 0, 2)
            npgb_tiles.append(np.ascontiguousarray(blk.reshape(P, 2 * g)))
            off += g

        # qgb tiles: [NQT, P, H2] with 120 graphs per 128-partition tile
        qgb_arr = np.zeros((NQT, P, H2), dtype=np.float32)
        qcore = qgb_all[gs:ge]
        for t in range(NQT):
            lo = t * GPT
            hi = min(lo + GPT, gcount)
            if lo < gcount:
                qgb_arr[t, : hi - lo] = qcore[lo:hi]
        qgb_c = np.ascontiguousarray(
            qgb_arr.astype(BF16_NP).transpose(1, 0, 2).reshape(P, NQT * H2)
        )

        # -qgb for the DVE-routed blocks, [h2dim, routed-graph] layout
        nqgb_arr = np.zeros((N_ROUTED * GPB, H2), dtype=np.float32)
        for r, b in enumerate(ROUTED):
            lo = b * GPB
            hi = min(lo + GPB, gcount)
            if lo < gcount:
                nqgb_arr[r * GPB : r * GPB + hi - lo] = -qcore[lo:hi]
        nqgb_c = np.ascontiguousarray(nqgb_arr.astype(BF16_NP).T)

        m = {
            "xt": xt_c,
            "qgb": qgb_c,
            "nqgb": nqgb_c,
            "sel": sel_bf,
            "w1a": w1a,
            "w2": w2bf,
            "w3": w3,
            "b3": b3r,
        }
        for k, t in enumerate(npgb_tiles):
            m[f"npgb{k}"] = t
        in_maps.append(m)

    # host-side correction vector: routed graphs get their dropped
    # qgb@W3 term back (b3 is applied on-device for all nodes)
    corr = np.zeros(N_GRAPHS, dtype=np.float32)
    for c in range(N_CORES):
        gs, ge = int(bounds[c]), int(bounds[c + 1])
        local = np.arange(ge - gs)
        routed_set = np.zeros(BLOCKS, dtype=bool)
        routed_set[ROUTED] = True
        corr[gs:ge] = np.where(routed_set[local // GPB], rgb_all[gs:ge], 0.0)
    return in_maps, counts, corr


LAST_RESULTS = None  # BassKernelResults of the most recent kernel() call


def kernel(**inputs) -> np.ndarray:
    global LAST_RESULTS
    if not _uniform_structure(inputs["node_to_graphid"], inputs["graph_offsets"]):
        # Structure differs from the oracle's fixed layout (40 nodes/graph,
        # offsets = 40*g); fall back to a straight host computation.
        return _reference_numpy(**inputs)

    in_maps, counts, corr = make_in_maps(inputs)
    nc = _get_program()
    res = run_bass_kernel_spmd(nc, in_maps, core_ids=list(range(N_CORES)))
    LAST_RESULTS = res
    pieces = []
    for c in range(N_CORES):
        flat = res.results[c]["out"].reshape(-1)
        pieces.append(flat[: NPG * counts[c]])
    full = np.concatenate(pieces) + np.repeat(corr, NPG)
    return full.reshape(N_NODES, 1).astype(np.float32)


if __name__ == "__main__":
    # smoke-trace the program without running it
    prog = _get_program()
    print("traced OK:", len(prog.m.functions[0].instructions)
          if hasattr(prog.m.functions[0], "instructions") else "n/a")
